# revision 17
# baseline (speedup 1.0000x reference)
"""Bass/Trainium2 kernel for nn_DeepIRTModel (DKVMN knowledge tracing).

Data-parallel over batch (B=256 -> 32 per core on 8 cores). Scan state
X = Mv lives in SBUF as (128 partitions = 32*g+b, free = (j,d)) with slot
n = 13*g + j (52 padded slots, pad weights = 0), d = DV = 200.

Per scan step the update is X <- X (.) A_t + B_t with
  A_t = 1 - w_t (x) er_t   (13 slices: (er * -w_j) + 1, tensor_scalar 4x)
  B_t = w_t (x) ad_t       (13 slices: ad * w_j)
A/B slices have no dependence on X, so they are built one step ahead and
distributed across ACT/GPSIMD/DVE; DVE's per-step critical work is just
two full-state tensor_tensor passes (mult + add).  Reads (PE, 26 matmuls
with block-diag wsel) use X before the update via Tile WAR deps.
Phase C (gathers/softmax/er/ad) and phase E (theta/beta/pred) are
emitted interleaved with scan chunks to keep engines busy.
"""
import sys, types

sys.path.insert(0, '/opt/trn_rl_repo')
import numpy as np
import ml_dtypes


def _install_ntff_hook():
    try:
        import antenv
        if "antenv.axon_hooks" in sys.modules:
            return
        mod = types.ModuleType("antenv.axon_hooks")
        state = {"hook": None}
        mod.set_axon_ntff_profile_hook = lambda h: state.__setitem__("hook", h)
        mod.get_axon_ntff_profile_hook = lambda: state["hook"]
        sys.modules["antenv.axon_hooks"] = mod
        antenv.axon_hooks = mod
        from trn_agent_boot.trn_boot import _ntff_profile_via_ctypes
        mod.set_axon_ntff_profile_hook(_ntff_profile_via_ctypes('/opt/axon/libaxon_pjrt.so'))
    except Exception:
        pass


_install_ntff_hook()

import concourse.bass as bass
import concourse.bacc as bacc
import concourse.mybir as mybir
from concourse.tile import TileContext, add_dep_helper
from concourse.bass_utils import run_bass_kernel_spmd

F16 = mybir.dt.float16
F32 = mybir.dt.float32
I32 = mybir.dt.int32
AF = mybir.ActivationFunctionType
OP = mybir.AluOpType

NQ, M, DK, DV, DS = 50000, 50, 50, 200, 50
B, T_FULL = 256, 200
SCALE = 3.0
NCORES = 8
BL = B // NCORES          # 32 batch rows per core
NJ = 13                   # slot groups per partition-subindex g in [0,4)
MP = 4 * NJ               # 52 padded slots
FREE = NJ * DV            # 2600 state free dim

# A/B slice -> engine assignment (tunable). 26 slices total per step.
A_ACT = (0, 1, 2, 3)      # A slices on ACT
B_ACT = (0, 1)            # B slices on ACT
B_GPS = (2, 3, 4, 5, 6)   # B slices on GPSIMD
# remainder on DVE
A_DVE = tuple(j for j in range(NJ) if j not in A_ACT)
B_DVE = tuple(j for j in range(NJ) if j not in B_ACT and j not in B_GPS)

CH_SCHED = [4, 8, 13] + [25] * 7      # chunk lengths, sums to 200


def build_nc(T=T_FULL, sim_safe=False):
    R = BL * T            # samples per core
    NCH = R // 128        # gather chunks of 128 rows
    assert R % 128 == 0
    sched = list(CH_SCHED)
    assert sum(sched) == T
    nchunks = len(sched)
    t_start = [sum(sched[:i]) for i in range(nchunks)]
    t_end = [sum(sched[:i + 1]) for i in range(nchunks)]
    k_need = [min(NCH, (t_end[i] + 3) // 4) for i in range(nchunks)]
    NSEG = (R + 511) // 512

    nc = bacc.Bacc(trn_type="TRN2")
    # ---- DRAM I/O ----
    qi_d = nc.dram_tensor("qi", [R], I32, kind="ExternalInput")
    qai_d = nc.dram_tensor("qai", [R], I32, kind="ExternalInput")
    qtab_d = nc.dram_tensor("qtab", [NQ + 1, DK], F32, kind="ExternalInput")
    qatab_d = nc.dram_tensor("qatab", [2 * NQ + 1, DV], F32, kind="ExternalInput")
    x0_d = nc.dram_tensor("x0", [128, FREE], F16, kind="ExternalInput")
    isel_d = nc.dram_tensor("isel", [128, BL], F16, kind="ExternalInput")
    ident_d = nc.dram_tensor("ident", [128, 128], F32, kind="ExternalInput")
    mkt_d = nc.dram_tensor("mkt", [DK, M], F16, kind="ExternalInput")
    we1_d = nc.dram_tensor("we1", [128, DV], F16, kind="ExternalInput")
    we2_d = nc.dram_tensor("we2", [73, DV], F16, kind="ExternalInput")
    wad1_d = nc.dram_tensor("wad1", [128, DV], F16, kind="ExternalInput")
    wad2_d = nc.dram_tensor("wad2", [73, DV], F16, kind="ExternalInput")
    wd1_d = nc.dram_tensor("wd1", [DK + 1, DS], F16, kind="ExternalInput")
    wd2_d = nc.dram_tensor("wd2", [DS + 1, 1], F16, kind="ExternalInput")
    ws1_d = nc.dram_tensor("ws1", [128, DS], F16, kind="ExternalInput")
    ws2_d = nc.dram_tensor("ws2", [72, DS], F16, kind="ExternalInput")
    ws3_d = nc.dram_tensor("ws3", [DK + 1, DS], F16, kind="ExternalInput")
    wa1_d = nc.dram_tensor("wa1", [DS + 1, DS], F16, kind="ExternalInput")
    wa2_d = nc.dram_tensor("wa2", [DS + 1, 1], F16, kind="ExternalInput")
    ones_d = nc.dram_tensor("ones", [1, BL * T], F16, kind="ExternalInput")
    pred_d = nc.dram_tensor("pred", [BL, T], F32, kind="ExternalOutput")
    # internal DRAM roundtrip buffers
    er_dram = nc.dram_tensor("er_dram", [R, DV], F16)
    ad_dram = nc.dram_tensor("ad_dram", [R, DV], F16)
    w_dram = nc.dram_tensor("w_dram", [R, MP], F32)

    with TileContext(nc) as tc:
        with tc.tile_pool(name="pers", bufs=1) as pers, \
             tc.tile_pool(name="work", bufs=2) as work, \
             tc.tile_pool(name="ac_sb", bufs=1) as ac_sb, \
             tc.tile_pool(name="ac_ps", bufs=2, space="PSUM") as ac_ps, \
             tc.tile_pool(name="d_ps", bufs=2, space="PSUM") as d_ps, \
             tc.tile_pool(name="e_ps", bufs=1, space="PSUM") as e_ps:

            # ---------- phase A: params + indices ----------
            ident = pers.tile([128, 128], F32)
            nc.sync.dma_start(out=ident, in_=ident_d[:, :])
            iselb = pers.tile([128, BL], F16)
            nc.sync.dma_start(out=iselb, in_=isel_d[:, :])
            x = pers.tile([128, FREE], F16)
            nc.sync.dma_start(out=x, in_=x0_d[:, :])
            mkt = pers.tile([DK, M], F16)
            nc.sync.dma_start(out=mkt, in_=mkt_d[:, :])
            wd1 = pers.tile([DK + 1, DS], F16)
            nc.sync.dma_start(out=wd1, in_=wd1_d[:, :])
            wd2 = pers.tile([DS + 1, 1], F16)
            nc.sync.dma_start(out=wd2, in_=wd2_d[:, :])
            ws1 = pers.tile([128, DS], F16)
            nc.sync.dma_start(out=ws1, in_=ws1_d[:, :])
            ws2 = pers.tile([72, DS], F16)
            nc.sync.dma_start(out=ws2, in_=ws2_d[:, :])
            ws3 = pers.tile([DK + 1, DS], F16)
            nc.sync.dma_start(out=ws3, in_=ws3_d[:, :])
            wa1 = pers.tile([DS + 1, DS], F16)
            nc.sync.dma_start(out=wa1, in_=wa1_d[:, :])
            wa2 = pers.tile([DS + 1, 1], F16)
            nc.sync.dma_start(out=wa2, in_=wa2_d[:, :])
            we1 = pers.tile([128, DV], F16)
            nc.sync.dma_start(out=we1, in_=we1_d[:, :])
            we2 = pers.tile([73, DV], F16)
            nc.sync.dma_start(out=we2, in_=we2_d[:, :])
            wad1 = pers.tile([128, DV], F16)
            nc.sync.dma_start(out=wad1, in_=wad1_d[:, :])
            wad2 = pers.tile([73, DV], F16)
            nc.sync.dma_start(out=wad2, in_=wad2_d[:, :])

            qi_sb = pers.tile([128, NCH], I32)
            nc.sync.dma_start(out=qi_sb, in_=qi_d.rearrange("(k p) -> p k", p=128))
            qai_sb = pers.tile([128, NCH], I32)
            nc.sync.dma_start(out=qai_sb, in_=qai_d.rearrange("(k p) -> p k", p=128))

            # persistent across phases
            qeT = pers.tile([DK + 1, R], F16)
            nc.sync.dma_start(out=qeT[DK:DK + 1, :], in_=ones_d[:, :R])
            qaeT_lo = pers.tile([128, R], F16)
            qaeT_hi = pers.tile([73, R], F16)
            nc.sync.dma_start(out=qaeT_hi[72:73, :], in_=ones_d[:, :R])
            beta_sb = pers.tile([128, NCH], F32)
            th_sb = pers.tile([128, NCH], F32)
            w_scan = pers.tile([128, T * NJ], F32)
            negw = pers.tile([128, T * NJ], F32)
            w_scanb = pers.tile([128, T * NJ], F16)
            readT_lo = pers.tile([128, R], F16)
            readT_hi = pers.tile([72, R], F16)
            zpad = pers.tile([128, 2], F32)
            nc.vector.memset(zpad, 0.0)

            er_src = er_dram.rearrange("(t b) d -> t b d", b=BL)
            ad_src = ad_dram.rearrange("(t b) d -> t b d", b=BL)
            w_w_insts = {}
            er_w_insts = {}
            ad_w_insts = {}

            # ---------- phase C: one gather-group (128 samples) ----------
            def emit_group(k):
                qe_g = ac_sb.tile([128, DK], F32, tag="qe_g", bufs=3)
                nc.gpsimd.indirect_dma_start(
                    out=qe_g, out_offset=None, in_=qtab_d[:, :],
                    in_offset=bass.IndirectOffsetOnAxis(
                        ap=qi_sb[:, k:k + 1], axis=0))
                qae_g = ac_sb.tile([128, DV], F32, tag="qae_g", bufs=3)
                nc.gpsimd.indirect_dma_start(
                    out=qae_g, out_offset=None, in_=qatab_d[:, :],
                    in_offset=bass.IndirectOffsetOnAxis(
                        ap=qai_sb[:, k:k + 1], axis=0))
                pt = ac_ps.tile([128, 128], F32, tag="pt", bufs=1)
                nc.tensor.transpose(out=pt[:DK, :], in_=qe_g, identity=ident)
                nc.scalar.copy(qeT[:DK, 128 * k:128 * (k + 1)], pt[:DK, :])
                pt2 = ac_ps.tile([128, 128], F32, tag="pt", bufs=1)
                nc.tensor.transpose(out=pt2, in_=qae_g[:, :128], identity=ident)
                nc.scalar.copy(qaeT_lo[:, 128 * k:128 * (k + 1)], pt2)
                pt3 = ac_ps.tile([128, 128], F32, tag="pt", bufs=1)
                nc.tensor.transpose(out=pt3[:72, :], in_=qae_g[:, 128:200],
                                    identity=ident)
                nc.scalar.copy(qaeT_hi[:72, 128 * k:128 * (k + 1)], pt3[:72, :])
                # softmax over memory slots -> w
                lg = ac_ps.tile([128, M], F32, tag="lg", bufs=1)
                nc.tensor.matmul(lg, lhsT=qeT[:DK, 128 * k:128 * (k + 1)],
                                 rhs=mkt, start=True, stop=True)
                ex = ac_sb.tile([128, M], F32, tag="ex", bufs=2)
                sm = ac_sb.tile([128, 1], F32, tag="sm", bufs=2)
                nc.scalar.activation(out=ex, in_=lg, func=AF.Exp, accum_out=sm)
                rc = ac_sb.tile([128, 1], F32, tag="rc", bufs=2)
                nc.vector.reciprocal(rc, sm)
                wn = ac_sb.tile([128, M], F32, tag="wn", bufs=2)
                nc.scalar.activation(out=wn, in_=ex, func=AF.Copy,
                                     scale=rc[:, 0:1])
                w_w_insts[2 * k] = nc.sync.dma_start(
                    out=w_dram[128 * k:128 * (k + 1), :M], in_=wn)
                w_w_insts[2 * k + 1] = nc.sync.dma_start(
                    out=w_dram[128 * k:128 * (k + 1), M:MP], in_=zpad[:, :2])
                # erase gate
                ep = ac_ps.tile([128, DV], F32, tag="eap", bufs=2, name="ep")
                nc.tensor.matmul(ep, lhsT=qaeT_lo[:, 128 * k:128 * (k + 1)],
                                 rhs=we1, start=True, stop=False)
                nc.tensor.matmul(ep, lhsT=qaeT_hi[:, 128 * k:128 * (k + 1)],
                                 rhs=we2, start=False, stop=True)
                ero = ac_sb.tile([128, DV], F16, tag="ero", bufs=2)
                nc.scalar.activation(out=ero, in_=ep, func=AF.Sigmoid)
                er_w_insts[k] = nc.sync.dma_start(
                    out=er_dram[128 * k:128 * (k + 1), :], in_=ero)
                # add vector
                ap_ = ac_ps.tile([128, DV], F32, tag="eap", bufs=2, name="ap_")
                nc.tensor.matmul(ap_, lhsT=qaeT_lo[:, 128 * k:128 * (k + 1)],
                                 rhs=wad1, start=True, stop=False)
                nc.tensor.matmul(ap_, lhsT=qaeT_hi[:, 128 * k:128 * (k + 1)],
                                 rhs=wad2, start=False, stop=True)
                ado = ac_sb.tile([128, DV], F16, tag="ado", bufs=2)
                nc.scalar.activation(out=ado, in_=ap_, func=AF.Tanh)
                ad_w_insts[k] = nc.sync.dma_start(
                    out=ad_dram[128 * k:128 * (k + 1), :], in_=ado)

            # ---------- chunk loads (er/ad/w -> batch-partition layout) ----
            er_tiles = {}
            ad_tiles = {}

            def emit_loads(ch):
                t0, t1 = t_start[ch], t_end[ch]
                tl = t1 - t0
                ks = range(t0 // 4, k_need[ch])
                er_ch = work.tile([128, 25 * DV], F16, tag="er_ch", bufs=2)
                ad_ch = work.tile([128, 25 * DV], F16, tag="ad_ch", bufs=2)
                er_tiles[ch] = er_ch
                ad_tiles[ch] = ad_ch
                if sim_safe:
                    nc.vector.memset(er_ch, 0.0)
                    nc.vector.memset(ad_ch, 0.0)
                for g in range(4):
                    ldi = nc.sync.dma_start(
                        out=er_ch[32 * g:32 * (g + 1), :tl * DV].rearrange(
                            "b (t d) -> b t d", d=DV),
                        in_=er_src[t0:t1, :, :].rearrange("t b d -> b t d"))
                    for k in ks:
                        add_dep_helper(ldi.ins, er_w_insts[k].ins,
                                       reason="er roundtrip")
                    lda = nc.gpsimd.dma_start(
                        out=ad_ch[32 * g:32 * (g + 1), :tl * DV].rearrange(
                            "b (t d) -> b t d", d=DV),
                        in_=ad_src[t0:t1, :, :].rearrange("t b d -> b t d"))
                    for k in ks:
                        add_dep_helper(lda.ins, ad_w_insts[k].ins,
                                       reason="ad roundtrip")
                if sim_safe and ch == 0:
                    nc.vector.memset(w_scan, 0.0)
                for g in range(4):
                    src = w_dram[:, NJ * g:NJ * (g + 1)].rearrange(
                        "(t b) j -> b t j", b=BL)[:, t0:t1, :]
                    dst = w_scan[32 * g:32 * (g + 1),
                                 t0 * NJ:t1 * NJ].rearrange(
                        "b (t j) -> b t j", j=NJ)
                    ldw = nc.sync.dma_start(out=dst, in_=src)
                    for k in ks:
                        add_dep_helper(ldw.ins, w_w_insts[2 * k].ins,
                                       reason="w roundtrip")
                        add_dep_helper(ldw.ins, w_w_insts[2 * k + 1].ins,
                                       reason="w pad roundtrip")
                nc.vector.tensor_scalar(
                    out=negw[:, t0 * NJ:t1 * NJ], in0=w_scan[:, t0 * NJ:t1 * NJ],
                    scalar1=-1.0, scalar2=None, op0=OP.mult)
                nc.vector.tensor_copy(w_scanb[:, t0 * NJ:t1 * NJ],
                                      w_scan[:, t0 * NJ:t1 * NJ])

            # ---------- A/B build for one step (t), split across engines ---
            def which_chunk(t):
                for c in range(nchunks):
                    if t < t_end[c]:
                        return c
                return None

            def emit_ab(t):
                ch = which_chunk(t)
                tt = t - t_start[ch]
                er_t = er_tiles[ch][:, tt * DV:(tt + 1) * DV]
                ad_t = ad_tiles[ch][:, tt * DV:(tt + 1) * DV]
                a_t = work.tile([128, FREE], F16, tag="a_t", bufs=2)
                b_t = work.tile([128, FREE], F16, tag="b_t", bufs=2)
                for j in A_DVE:
                    nc.vector.tensor_scalar(
                        out=a_t[:, j * DV:(j + 1) * DV], in0=er_t,
                        scalar1=negw[:, t * NJ + j:t * NJ + j + 1],
                        scalar2=1.0, op0=OP.mult, op1=OP.add)
                for j in B_DVE:
                    nc.vector.tensor_scalar(
                        out=b_t[:, j * DV:(j + 1) * DV], in0=ad_t,
                        scalar1=w_scan[:, t * NJ + j:t * NJ + j + 1],
                        scalar2=None, op0=OP.mult)
                for j in A_ACT:
                    nc.scalar.activation(
                        out=a_t[:, j * DV:(j + 1) * DV], in_=er_t, func=AF.Copy,
                        scale=negw[:, t * NJ + j:t * NJ + j + 1], bias=1.0)
                for j in B_ACT:
                    nc.scalar.activation(
                        out=b_t[:, j * DV:(j + 1) * DV], in_=ad_t, func=AF.Copy,
                        scale=w_scan[:, t * NJ + j:t * NJ + j + 1])
                for j in B_GPS:
                    nc.gpsimd.tensor_scalar(
                        out=b_t[:, j * DV:(j + 1) * DV], in0=ad_t,
                        scalar1=w_scan[:, t * NJ + j:t * NJ + j + 1],
                        scalar2=None, op0=OP.mult)
                return a_t, b_t

            def emit_wsel(t):
                wsel = work.tile([128, NJ * BL], F16, tag="wsel", bufs=2)
                nc.gpsimd.tensor_tensor(
                    out=wsel.rearrange("p (j b) -> p j b", j=NJ),
                    in0=w_scanb[:, t * NJ:(t + 1) * NJ].rearrange(
                        "p (j o) -> p j o", o=1).to_broadcast([128, NJ, BL]),
                    in1=iselb.rearrange("p (o b) -> p o b", o=1).to_broadcast(
                        [128, NJ, BL]),
                    op=OP.mult)
                return wsel

            # ---------- phase E: one 512-sample segment ----------
            def emit_seg(s):
                n0, n1 = 512 * s, min(512 * (s + 1), R)
                w_ = n1 - n0
                nk = w_ // 128
                h2 = work.tile([DS + 1, 512], F16, tag="h2", bufs=2)
                if s < 2:
                    nc.sync.dma_start(out=h2[DS:DS + 1, :], in_=ones_d[:, :512])
                hp = e_ps.tile([DS, 512], F32, tag="ep5", bufs=1)
                nc.tensor.matmul(hp[:, :w_], lhsT=wd1, rhs=qeT[:, n0:n1],
                                 start=True, stop=True)
                nc.scalar.activation(out=h2[:DS, :w_], in_=hp[:, :w_],
                                     func=AF.Tanh)
                bp = e_ps.tile([128, 4], F32, tag="col", bufs=1)
                for kk in range(nk):
                    nc.tensor.matmul(bp[:, kk:kk + 1],
                                     lhsT=h2[:, 128 * kk:128 * (kk + 1)],
                                     rhs=wd2, start=True, stop=True)
                nc.scalar.copy(beta_sb[:, 4 * s:4 * s + nk], bp[:, :nk])
                sm2 = work.tile([DS + 1, 512], F16, tag="sm2", bufs=2)
                if s < 2:
                    nc.sync.dma_start(out=sm2[DS:DS + 1, :], in_=ones_d[:, :512])
                sp = e_ps.tile([DS, 512], F32, tag="ep5", bufs=1)
                nc.tensor.matmul(sp[:, :w_], lhsT=ws1, rhs=readT_lo[:, n0:n1],
                                 start=True, stop=False)
                nc.tensor.matmul(sp[:, :w_], lhsT=ws2, rhs=readT_hi[:72, n0:n1],
                                 start=False, stop=False)
                nc.tensor.matmul(sp[:, :w_], lhsT=ws3, rhs=qeT[:, n0:n1],
                                 start=False, stop=True)
                nc.scalar.activation(out=sm2[:DS, :w_], in_=sp[:, :w_],
                                     func=AF.Tanh)
                ht = work.tile([DS + 1, 512], F16, tag="ht", bufs=2)
                if s < 2:
                    nc.sync.dma_start(out=ht[DS:DS + 1, :], in_=ones_d[:, :512])
                hp2 = e_ps.tile([DS, 512], F32, tag="ep5", bufs=1)
                nc.tensor.matmul(hp2[:, :w_], lhsT=wa1, rhs=sm2[:, :w_],
                                 start=True, stop=True)
                nc.scalar.activation(out=ht[:DS, :w_], in_=hp2[:, :w_],
                                     func=AF.Tanh)
                tp_ = e_ps.tile([128, 4], F32, tag="col", bufs=1)
                for kk in range(nk):
                    nc.tensor.matmul(tp_[:, kk:kk + 1],
                                     lhsT=ht[:, 128 * kk:128 * (kk + 1)],
                                     rhs=wa2, start=True, stop=True)
                nc.scalar.copy(th_sb[:, 4 * s:4 * s + nk], tp_[:, :nk])

            # ---------- one scan step ----------
            def emit_step(t, ab_cur, wsel_cur):
                # A/B + wsel for the NEXT step (no dependence on x)
                nxt = None
                if t + 1 < T:
                    nxt = (emit_ab(t + 1), emit_wsel(t + 1))
                a_t, b_t = ab_cur
                # PE: reads (use X before this step's update; Tile WAR deps
                # order these before the x writes below)
                rr = d_ps.tile([128, 2 * BL], F32, tag="rr", bufs=2)
                for j in range(NJ):
                    nc.tensor.matmul(rr[:, :BL], lhsT=x[:, j * DV:j * DV + 128],
                                     rhs=wsel_cur[:, j * BL:(j + 1) * BL],
                                     start=(j == 0), stop=(j == NJ - 1))
                for j in range(NJ):
                    nc.tensor.matmul(rr[:72, BL:2 * BL],
                                     lhsT=x[:, j * DV + 128:(j + 1) * DV],
                                     rhs=wsel_cur[:, j * BL:(j + 1) * BL],
                                     start=(j == 0), stop=(j == NJ - 1))
                nc.scalar.copy(readT_lo[:, BL * t:BL * (t + 1)], rr[:, :BL])
                nc.scalar.copy(readT_hi[:, BL * t:BL * (t + 1)],
                               rr[:72, BL:2 * BL])
                # DVE: the state update
                nc.vector.tensor_tensor(out=x, in0=x, in1=a_t, op=OP.mult)
                nc.vector.tensor_tensor(out=x, in0=x, in1=b_t, op=OP.add)
                return nxt

            # ---------- schedule ----------
            # upfront: groups for chunk 0+1, loads for chunk 0, first A/B.
            for k in range(k_need[1]):
                emit_group(k)
            emit_loads(0)
            emit_loads(1)
            ab = emit_ab(0)
            ws_ = emit_wsel(0)
            kdone = k_need[1]
            next_seg = 0
            for ch in range(nchunks):
                # groups for chunk ch+2 (already have <= ch+1)
                gk = list(range(kdone, k_need[ch + 2])) if ch + 2 < nchunks else []
                kdone = k_need[ch + 2] if ch + 2 < nchunks else kdone
                steps = list(range(t_start[ch], t_end[ch]))
                # spread group emission across the chunk's steps
                gpos = {}
                for i, k in enumerate(gk):
                    gpos.setdefault(int((i + 1) * len(steps) / (len(gk) + 1)), []).append(k)
                for si, t in enumerate(steps):
                    nxt = emit_step(t, ab, ws_)
                    if nxt is not None:
                        ab, ws_ = nxt
                    for k in gpos.get(si, []):
                        emit_group(k)
                # loads for chunk ch+2 emitted after its groups
                if ch + 2 < nchunks:
                    emit_loads(ch + 2)
                while next_seg < NSEG and 16 * (next_seg + 1) <= t_end[ch]:
                    emit_seg(next_seg)
                    next_seg += 1
            while next_seg < NSEG:
                emit_seg(next_seg)
                next_seg += 1

            # ---------- final: pred ----------
            pre = pers.tile([128, NCH], F32)
            nc.vector.scalar_tensor_tensor(out=pre, in0=th_sb, scalar=SCALE,
                                           in1=beta_sb, op0=OP.mult,
                                           op1=OP.subtract)
            pred_sb = pers.tile([128, NCH], F32)
            nc.scalar.activation(out=pred_sb, in_=pre, func=AF.Sigmoid)
            # pred_sb[p=32u+i, k] -> pred_d[b=i, t=4k+u]
            nc.sync.dma_start(
                out=pred_d.rearrange("b (k u) -> u b k", u=4),
                in_=pred_sb[:, :])

    return nc


_NC_CACHE = {}


def _get_nc(T=T_FULL):
    if T not in _NC_CACHE:
        n = build_nc(T=T)
        n.compile()
        _NC_CACHE[T] = n
    return _NC_CACHE[T]


def make_inmaps(q_data, qa_data, q_tab, qa_tab, Mk, Mv0, Ws, bs, Wa1, ba1, Wa2,
                ba2, Wd1, bd1, Wd2, bd2, We, be, Wad, bad, T=T_FULL):
    hf = np.float16
    f32 = np.float32
    q_data = np.asarray(q_data)
    qa_data = np.asarray(qa_data)

    mv0p = np.zeros((MP, DV), f32)
    mv0p[:M] = np.asarray(Mv0, f32)
    x0 = mv0p.reshape(4, NJ, DV)[:, None].repeat(BL, 1).reshape(128, FREE).astype(hf)
    isel = np.tile(np.eye(BL, dtype=f32), (4, 1)).astype(hf)  # (128, 32), g-major
    ident = np.eye(128, dtype=f32)

    cat = np.concatenate
    common = {
        "qtab": np.ascontiguousarray(np.asarray(q_tab, f32)),
        "qatab": np.ascontiguousarray(np.asarray(qa_tab, f32)),
        "x0": x0, "isel": isel, "ident": ident,
        "ones": np.ones((1, BL * T), f32).astype(hf),
        "mkt": np.asarray(Mk, f32).T.copy().astype(hf),
        "we1": np.asarray(We, f32)[:128].astype(hf),
        "we2": cat([np.asarray(We, f32)[128:], np.asarray(be, f32)[None, :]], 0).astype(hf),
        "wad1": np.asarray(Wad, f32)[:128].astype(hf),
        "wad2": cat([np.asarray(Wad, f32)[128:], np.asarray(bad, f32)[None, :]], 0).astype(hf),
        "wd1": cat([np.asarray(Wd1, f32), np.asarray(bd1, f32)[None, :]], 0).astype(hf),
        "wd2": cat([np.asarray(Wd2, f32), np.asarray(bd2, f32)[None, :]], 0).astype(hf),
        "ws1": np.asarray(Ws, f32)[:128].astype(hf),
        "ws2": np.asarray(Ws, f32)[128:200].astype(hf),
        "ws3": cat([np.asarray(Ws, f32)[200:], np.asarray(bs, f32)[None, :]], 0).astype(hf),
        "wa1": cat([np.asarray(Wa1, f32), np.asarray(ba1, f32)[None, :]], 0).astype(hf),
        "wa2": cat([np.asarray(Wa2, f32), np.asarray(ba2, f32)[None, :]], 0).astype(hf),
    }
    in_maps = []
    for c in range(NCORES):
        sl = slice(BL * c, BL * (c + 1))
        in_maps.append(dict(
            common,
            qi=np.ascontiguousarray(q_data[sl, :T].T).reshape(-1).astype(np.int32),
            qai=np.ascontiguousarray(qa_data[sl, :T].T).reshape(-1).astype(np.int32)))
    return in_maps


def kernel(**inputs):
    nc = _get_nc(T_FULL)
    in_maps = make_inmaps(**inputs)
    res = run_bass_kernel_spmd(nc, in_maps, core_ids=list(range(NCORES)), trace=False)
    return np.concatenate([res.results[c]["pred"] for c in range(NCORES)], axis=0)


# revision 25
# speedup vs baseline: 2.7160x; 2.7160x over previous
"""Bass/Trainium2 kernel for nn_DeepIRTModel (DKVMN knowledge tracing).

Data-parallel over batch (B=256 -> 32 per core on 8 cores). Scan state
X = Mv lives in SBUF as (128 partitions = 32*g+b, free = (j,d)) with slot
n = 13*g + j (52 padded slots, pad weights = 0), d = DV = 200.

Per scan step the update is X <- X (.) A_t + B_t with
  A_t = 1 - w_t (x) er_t   (13 slices: (er * -w_j) + 1, tensor_scalar 4x)
  B_t = w_t (x) ad_t       (13 slices: ad * w_j)
A/B slices have no dependence on X, so they are built one step ahead and
distributed across ACT/GPSIMD/DVE; DVE's per-step critical work is just
two full-state tensor_tensor passes (mult + add).  Reads (PE, 26 matmuls
with block-diag wsel) use X before the update via Tile WAR deps.
Phase C (gathers/softmax/er/ad) and phase E (theta/beta/pred) are
emitted interleaved with scan chunks to keep engines busy.
"""
import sys, types

sys.path.insert(0, '/opt/trn_rl_repo')
import numpy as np
import ml_dtypes


def _install_ntff_hook():
    try:
        import antenv
        if "antenv.axon_hooks" in sys.modules:
            return
        mod = types.ModuleType("antenv.axon_hooks")
        state = {"hook": None}
        mod.set_axon_ntff_profile_hook = lambda h: state.__setitem__("hook", h)
        mod.get_axon_ntff_profile_hook = lambda: state["hook"]
        sys.modules["antenv.axon_hooks"] = mod
        antenv.axon_hooks = mod
        from trn_agent_boot.trn_boot import _ntff_profile_via_ctypes
        mod.set_axon_ntff_profile_hook(_ntff_profile_via_ctypes('/opt/axon/libaxon_pjrt.so'))
    except Exception:
        pass


_install_ntff_hook()

import concourse.bass as bass
import concourse.bacc as bacc
import concourse.mybir as mybir
from concourse.tile import TileContext, add_dep_helper
from concourse.bass_utils import run_bass_kernel_spmd

BF16 = mybir.dt.bfloat16
F32 = mybir.dt.float32
I32 = mybir.dt.int32
AF = mybir.ActivationFunctionType
OP = mybir.AluOpType

NQ, M, DK, DV, DS = 50000, 50, 50, 200, 50
B, T_FULL = 256, 200
SCALE = 3.0
NCORES = 8
BL = B // NCORES          # 32 batch rows per core
NJ = 13                   # slot groups per partition-subindex g in [0,4)
MP = 4 * NJ               # 52 padded slots
FREE = NJ * DV            # 2600 state free dim

# wG slice -> engine split (tunable): slices j < KACT go to ACT (computed
# from va, available early); the rest are DVE tensor_scalar 4x ops.
KACT = 6                  # va covers slots 0..KACT-1, vb the rest

CH_SCHED = [4, 8, 13] + [25] * 7      # chunk lengths, sums to 200


def build_nc(T=T_FULL, sim_safe=False):
    R = BL * T            # samples per core
    NCH = R // 128        # gather chunks of 128 rows
    assert R % 128 == 0
    sched = list(CH_SCHED)
    assert sum(sched) == T
    nchunks = len(sched)
    t_start = [sum(sched[:i]) for i in range(nchunks)]
    t_end = [sum(sched[:i + 1]) for i in range(nchunks)]
    k_need = [min(NCH, (t_end[i] + 3) // 4) for i in range(nchunks)]
    NSEG = (R + 511) // 512

    nc = bacc.Bacc(trn_type="TRN2")
    # ---- DRAM I/O ----
    qi_d = nc.dram_tensor("qi", [R], I32, kind="ExternalInput")
    qai_d = nc.dram_tensor("qai", [R], I32, kind="ExternalInput")
    qtab_d = nc.dram_tensor("qtab", [NQ + 1, DK], F32, kind="ExternalInput")
    qatab_d = nc.dram_tensor("qatab", [2 * NQ + 1, DV], F32, kind="ExternalInput")
    x0_d = nc.dram_tensor("x0", [128, FREE], BF16, kind="ExternalInput")
    isel_d = nc.dram_tensor("isel", [128, BL], BF16, kind="ExternalInput")
    ident_d = nc.dram_tensor("ident", [128, 128], F32, kind="ExternalInput")
    mkt_d = nc.dram_tensor("mkt", [DK, M], BF16, kind="ExternalInput")
    we1_d = nc.dram_tensor("we1", [128, DV], BF16, kind="ExternalInput")
    we2_d = nc.dram_tensor("we2", [73, DV], BF16, kind="ExternalInput")
    wad1_d = nc.dram_tensor("wad1", [128, DV], BF16, kind="ExternalInput")
    wad2_d = nc.dram_tensor("wad2", [73, DV], BF16, kind="ExternalInput")
    wd1_d = nc.dram_tensor("wd1", [DK + 1, DS], BF16, kind="ExternalInput")
    wd2_d = nc.dram_tensor("wd2", [DS + 1, 1], BF16, kind="ExternalInput")
    ws1_d = nc.dram_tensor("ws1", [128, DS], BF16, kind="ExternalInput")
    ws2_d = nc.dram_tensor("ws2", [72, DS], BF16, kind="ExternalInput")
    ws3_d = nc.dram_tensor("ws3", [DK + 1, DS], BF16, kind="ExternalInput")
    wa1_d = nc.dram_tensor("wa1", [DS + 1, DS], BF16, kind="ExternalInput")
    wa2_d = nc.dram_tensor("wa2", [DS + 1, 1], BF16, kind="ExternalInput")
    ones_d = nc.dram_tensor("ones", [1, BL * T], BF16, kind="ExternalInput")
    pred_d = nc.dram_tensor("pred", [BL, T], F32, kind="ExternalOutput")
    # internal DRAM roundtrip buffers
    er_dram = nc.dram_tensor("er_dram", [R, DV], BF16)
    ad_dram = nc.dram_tensor("ad_dram", [R, DV], BF16)
    w_dram = nc.dram_tensor("w_dram", [R, MP], F32)

    with TileContext(nc) as tc:
        with tc.tile_pool(name="pers", bufs=1) as pers, \
             tc.tile_pool(name="work", bufs=2) as work, \
             tc.tile_pool(name="ac_sb", bufs=1) as ac_sb, \
             tc.tile_pool(name="ac_ps", bufs=2, space="PSUM") as ac_ps, \
             tc.tile_pool(name="d_ps", bufs=2, space="PSUM") as d_ps, \
             tc.tile_pool(name="e_ps", bufs=1, space="PSUM") as e_ps:

            # ---------- phase A: params + indices ----------
            ident = pers.tile([128, 128], F32)
            nc.sync.dma_start(out=ident, in_=ident_d[:, :])
            iselb = pers.tile([128, BL], BF16)
            nc.sync.dma_start(out=iselb, in_=isel_d[:, :])
            x = pers.tile([128, FREE], BF16)
            nc.sync.dma_start(out=x, in_=x0_d[:, :])
            mkt = pers.tile([DK, M], BF16)
            nc.sync.dma_start(out=mkt, in_=mkt_d[:, :])
            wd1 = pers.tile([DK + 1, DS], BF16)
            nc.sync.dma_start(out=wd1, in_=wd1_d[:, :])
            wd2 = pers.tile([DS + 1, 1], BF16)
            nc.sync.dma_start(out=wd2, in_=wd2_d[:, :])
            ws1 = pers.tile([128, DS], BF16)
            nc.sync.dma_start(out=ws1, in_=ws1_d[:, :])
            ws2 = pers.tile([72, DS], BF16)
            nc.sync.dma_start(out=ws2, in_=ws2_d[:, :])
            ws3 = pers.tile([DK + 1, DS], BF16)
            nc.sync.dma_start(out=ws3, in_=ws3_d[:, :])
            wa1 = pers.tile([DS + 1, DS], BF16)
            nc.sync.dma_start(out=wa1, in_=wa1_d[:, :])
            wa2 = pers.tile([DS + 1, 1], BF16)
            nc.sync.dma_start(out=wa2, in_=wa2_d[:, :])
            we1 = pers.tile([128, DV], BF16)
            nc.sync.dma_start(out=we1, in_=we1_d[:, :])
            we2 = pers.tile([73, DV], BF16)
            nc.sync.dma_start(out=we2, in_=we2_d[:, :])
            wad1 = pers.tile([128, DV], BF16)
            nc.sync.dma_start(out=wad1, in_=wad1_d[:, :])
            wad2 = pers.tile([73, DV], BF16)
            nc.sync.dma_start(out=wad2, in_=wad2_d[:, :])

            qi_sb = pers.tile([128, NCH], I32)
            nc.sync.dma_start(out=qi_sb, in_=qi_d.rearrange("(k p) -> p k", p=128))
            qai_sb = pers.tile([128, NCH], I32)
            nc.sync.dma_start(out=qai_sb, in_=qai_d.rearrange("(k p) -> p k", p=128))

            # persistent across phases
            qeT = pers.tile([DK + 1, R], BF16)
            nc.sync.dma_start(out=qeT[DK:DK + 1, :], in_=ones_d[:, :R])
            qaeT_lo = pers.tile([128, R], BF16)
            qaeT_hi = pers.tile([73, R], BF16)
            nc.sync.dma_start(out=qaeT_hi[72:73, :], in_=ones_d[:, :R])
            beta_sb = pers.tile([128, NCH], F32)
            th_sb = pers.tile([128, NCH], F32)
            w_scan = pers.tile([128, T * NJ], F32)
            w_scanb = pers.tile([128, T * NJ], BF16)
            readT_lo = pers.tile([128, R], BF16)
            readT_hi = pers.tile([72, R], BF16)
            zpad = pers.tile([128, 2], F32)
            nc.vector.memset(zpad, 0.0)

            er_src = er_dram.rearrange("(t b) d -> t b d", b=BL)
            ad_src = ad_dram.rearrange("(t b) d -> t b d", b=BL)
            w_w_insts = {}
            er_w_insts = {}
            ad_w_insts = {}

            # ---------- phase C: one gather-group (128 samples) ----------
            def emit_group(k):
                qe_g = ac_sb.tile([128, DK], F32, tag="qe_g", bufs=3)
                nc.gpsimd.indirect_dma_start(
                    out=qe_g, out_offset=None, in_=qtab_d[:, :],
                    in_offset=bass.IndirectOffsetOnAxis(
                        ap=qi_sb[:, k:k + 1], axis=0))
                qae_g = ac_sb.tile([128, DV], F32, tag="qae_g", bufs=3)
                nc.gpsimd.indirect_dma_start(
                    out=qae_g, out_offset=None, in_=qatab_d[:, :],
                    in_offset=bass.IndirectOffsetOnAxis(
                        ap=qai_sb[:, k:k + 1], axis=0))
                pt = ac_ps.tile([128, 128], F32, tag="pt", bufs=1)
                nc.tensor.transpose(out=pt[:DK, :], in_=qe_g, identity=ident)
                nc.scalar.copy(qeT[:DK, 128 * k:128 * (k + 1)], pt[:DK, :])
                pt2 = ac_ps.tile([128, 128], F32, tag="pt", bufs=1)
                nc.tensor.transpose(out=pt2, in_=qae_g[:, :128], identity=ident)
                nc.scalar.copy(qaeT_lo[:, 128 * k:128 * (k + 1)], pt2)
                pt3 = ac_ps.tile([128, 128], F32, tag="pt", bufs=1)
                nc.tensor.transpose(out=pt3[:72, :], in_=qae_g[:, 128:200],
                                    identity=ident)
                nc.scalar.copy(qaeT_hi[:72, 128 * k:128 * (k + 1)], pt3[:72, :])
                # softmax over memory slots -> w
                lg = ac_ps.tile([128, M], F32, tag="lg", bufs=1)
                nc.tensor.matmul(lg, lhsT=qeT[:DK, 128 * k:128 * (k + 1)],
                                 rhs=mkt, start=True, stop=True)
                ex = ac_sb.tile([128, M], F32, tag="ex", bufs=2)
                sm = ac_sb.tile([128, 1], F32, tag="sm", bufs=2)
                nc.scalar.activation(out=ex, in_=lg, func=AF.Exp, accum_out=sm)
                rc = ac_sb.tile([128, 1], F32, tag="rc", bufs=2)
                nc.vector.reciprocal(rc, sm)
                wn = ac_sb.tile([128, M], F32, tag="wn", bufs=2)
                nc.scalar.activation(out=wn, in_=ex, func=AF.Copy,
                                     scale=rc[:, 0:1])
                w_w_insts[2 * k] = nc.sync.dma_start(
                    out=w_dram[128 * k:128 * (k + 1), :M], in_=wn)
                w_w_insts[2 * k + 1] = nc.sync.dma_start(
                    out=w_dram[128 * k:128 * (k + 1), M:MP], in_=zpad[:, :2])
                # erase gate
                ep = ac_ps.tile([128, DV], F32, tag="eap", bufs=2, name="ep")
                nc.tensor.matmul(ep, lhsT=qaeT_lo[:, 128 * k:128 * (k + 1)],
                                 rhs=we1, start=True, stop=False)
                nc.tensor.matmul(ep, lhsT=qaeT_hi[:, 128 * k:128 * (k + 1)],
                                 rhs=we2, start=False, stop=True)
                ero = ac_sb.tile([128, DV], BF16, tag="ero", bufs=2)
                nc.scalar.activation(out=ero, in_=ep, func=AF.Sigmoid)
                er_w_insts[k] = nc.sync.dma_start(
                    out=er_dram[128 * k:128 * (k + 1), :], in_=ero)
                # add vector
                ap_ = ac_ps.tile([128, DV], F32, tag="eap", bufs=2, name="ap_")
                nc.tensor.matmul(ap_, lhsT=qaeT_lo[:, 128 * k:128 * (k + 1)],
                                 rhs=wad1, start=True, stop=False)
                nc.tensor.matmul(ap_, lhsT=qaeT_hi[:, 128 * k:128 * (k + 1)],
                                 rhs=wad2, start=False, stop=True)
                ado = ac_sb.tile([128, DV], BF16, tag="ado", bufs=2)
                nc.scalar.activation(out=ado, in_=ap_, func=AF.Tanh)
                ad_w_insts[k] = nc.sync.dma_start(
                    out=ad_dram[128 * k:128 * (k + 1), :], in_=ado)

            # ---------- chunk loads (er/ad/w -> batch-partition layout) ----
            er_tiles = {}
            ad_tiles = {}

            def emit_loads(ch):
                t0, t1 = t_start[ch], t_end[ch]
                tl = t1 - t0
                ks = range(t0 // 4, k_need[ch])
                er_ch = work.tile([128, 25 * DV], BF16, tag="er_ch", bufs=2)
                ad_ch = work.tile([128, 25 * DV], BF16, tag="ad_ch", bufs=2)
                er_tiles[ch] = er_ch
                ad_tiles[ch] = ad_ch
                if sim_safe:
                    nc.vector.memset(er_ch, 0.0)
                    nc.vector.memset(ad_ch, 0.0)
                for g in range(4):
                    ldi = nc.sync.dma_start(
                        out=er_ch[32 * g:32 * (g + 1), :tl * DV].rearrange(
                            "b (t d) -> b t d", d=DV),
                        in_=er_src[t0:t1, :, :].rearrange("t b d -> b t d"))
                    for k in ks:
                        add_dep_helper(ldi.ins, er_w_insts[k].ins,
                                       reason="er roundtrip")
                    lda = nc.gpsimd.dma_start(
                        out=ad_ch[32 * g:32 * (g + 1), :tl * DV].rearrange(
                            "b (t d) -> b t d", d=DV),
                        in_=ad_src[t0:t1, :, :].rearrange("t b d -> b t d"))
                    for k in ks:
                        add_dep_helper(lda.ins, ad_w_insts[k].ins,
                                       reason="ad roundtrip")
                if sim_safe and ch == 0:
                    nc.vector.memset(w_scan, 0.0)
                for g in range(4):
                    src = w_dram[:, NJ * g:NJ * (g + 1)].rearrange(
                        "(t b) j -> b t j", b=BL)[:, t0:t1, :]
                    dst = w_scan[32 * g:32 * (g + 1),
                                 t0 * NJ:t1 * NJ].rearrange(
                        "b (t j) -> b t j", j=NJ)
                    ldw = nc.sync.dma_start(out=dst, in_=src)
                    for k in ks:
                        add_dep_helper(ldw.ins, w_w_insts[2 * k].ins,
                                       reason="w roundtrip")
                        add_dep_helper(ldw.ins, w_w_insts[2 * k + 1].ins,
                                       reason="w pad roundtrip")
                nc.vector.tensor_copy(w_scanb[:, t0 * NJ:t1 * NJ],
                                      w_scan[:, t0 * NJ:t1 * NJ])

            def which_chunk(t):
                for c in range(nchunks):
                    if t < t_end[c]:
                        return c
                return None

            def emit_wsel(t):
                wsel = work.tile([128, NJ * BL], BF16, tag="wsel", bufs=2)
                nc.gpsimd.tensor_tensor(
                    out=wsel.rearrange("p (j b) -> p j b", j=NJ),
                    in0=w_scanb[:, t * NJ:(t + 1) * NJ].rearrange(
                        "p (j o) -> p j o", o=1).to_broadcast([128, NJ, BL]),
                    in1=iselb.rearrange("p (o b) -> p o b", o=1).to_broadcast(
                        [128, NJ, BL]),
                    op=OP.mult)
                return wsel

            # ---------- phase E: one 512-sample segment ----------
            def emit_seg(s):
                n0, n1 = 512 * s, min(512 * (s + 1), R)
                w_ = n1 - n0
                nk = w_ // 128
                h2 = work.tile([DS + 1, 512], BF16, tag="h2", bufs=2)
                if s < 2:
                    nc.sync.dma_start(out=h2[DS:DS + 1, :], in_=ones_d[:, :512])
                hp = e_ps.tile([DS, 512], F32, tag="ep5", bufs=1)
                nc.tensor.matmul(hp[:, :w_], lhsT=wd1, rhs=qeT[:, n0:n1],
                                 start=True, stop=True)
                nc.scalar.activation(out=h2[:DS, :w_], in_=hp[:, :w_],
                                     func=AF.Tanh)
                bp = e_ps.tile([128, 4], F32, tag="col", bufs=1)
                for kk in range(nk):
                    nc.tensor.matmul(bp[:, kk:kk + 1],
                                     lhsT=h2[:, 128 * kk:128 * (kk + 1)],
                                     rhs=wd2, start=True, stop=True)
                nc.scalar.copy(beta_sb[:, 4 * s:4 * s + nk], bp[:, :nk])
                sm2 = work.tile([DS + 1, 512], BF16, tag="sm2", bufs=2)
                if s < 2:
                    nc.sync.dma_start(out=sm2[DS:DS + 1, :], in_=ones_d[:, :512])
                sp = e_ps.tile([DS, 512], F32, tag="ep5", bufs=1)
                nc.tensor.matmul(sp[:, :w_], lhsT=ws1, rhs=readT_lo[:, n0:n1],
                                 start=True, stop=False)
                nc.tensor.matmul(sp[:, :w_], lhsT=ws2, rhs=readT_hi[:72, n0:n1],
                                 start=False, stop=False)
                nc.tensor.matmul(sp[:, :w_], lhsT=ws3, rhs=qeT[:, n0:n1],
                                 start=False, stop=True)
                nc.scalar.activation(out=sm2[:DS, :w_], in_=sp[:, :w_],
                                     func=AF.Tanh)
                ht = work.tile([DS + 1, 512], BF16, tag="ht", bufs=2)
                if s < 2:
                    nc.sync.dma_start(out=ht[DS:DS + 1, :], in_=ones_d[:, :512])
                hp2 = e_ps.tile([DS, 512], F32, tag="ep5", bufs=1)
                nc.tensor.matmul(hp2[:, :w_], lhsT=wa1, rhs=sm2[:, :w_],
                                 start=True, stop=True)
                nc.scalar.activation(out=ht[:DS, :w_], in_=hp2[:, :w_],
                                     func=AF.Tanh)
                tp_ = e_ps.tile([128, 4], F32, tag="col", bufs=1)
                for kk in range(nk):
                    nc.tensor.matmul(tp_[:, kk:kk + 1],
                                     lhsT=ht[:, 128 * kk:128 * (kk + 1)],
                                     rhs=wa2, start=True, stop=True)
                nc.scalar.copy(th_sb[:, 4 * s:4 * s + nk], tp_[:, :nk])

            # ---------- one scan step ----------
            x3 = x.rearrange("p (j d) -> p j d", j=NJ)
            ka = KACT
            kb = NJ - KACT

            def emit_step(t, wsel_cur):
                nxt = emit_wsel(t + 1) if t + 1 < T else None
                ch = which_chunk(t)
                tt = t - t_start[ch]
                er_t = er_tiles[ch][:, tt * DV:(tt + 1) * DV]
                ad_t = ad_tiles[ch][:, tt * DV:(tt + 1) * DV]
                er_b = er_t.rearrange("p (o d) -> p o d", o=1).to_broadcast(
                    [128, NJ, DV])
                ad_b = ad_t.rearrange("p (o d) -> p o d", o=1).to_broadcast(
                    [128, NJ, DV])
                # PE: reads (use X before this step's update; Tile WAR deps
                # order these before the x writes below)
                rr = d_ps.tile([128, 2 * BL], F32, tag="rr", bufs=2)
                for j in range(NJ):
                    nc.tensor.matmul(rr[:, :BL], lhsT=x[:, j * DV:j * DV + 128],
                                     rhs=wsel_cur[:, j * BL:(j + 1) * BL],
                                     start=(j == 0), stop=(j == NJ - 1))
                for j in range(NJ):
                    nc.tensor.matmul(rr[:72, BL:2 * BL],
                                     lhsT=x[:, j * DV + 128:(j + 1) * DV],
                                     rhs=wsel_cur[:, j * BL:(j + 1) * BL],
                                     start=(j == 0), stop=(j == NJ - 1))
                nc.scalar.copy(readT_lo[:, BL * t:BL * (t + 1)], rr[:, :BL])
                nc.scalar.copy(readT_hi[:, BL * t:BL * (t + 1)],
                               rr[:72, BL:2 * BL])
                # DVE: V = X*er - ad (va early so ACT wG can start)
                va = work.tile([128, ka * DV], BF16, tag="va", bufs=2)
                vb = work.tile([128, kb * DV], BF16, tag="vb", bufs=2)
                va3 = va.rearrange("p (j d) -> p j d", j=ka)
                vb3 = vb.rearrange("p (j d) -> p j d", j=kb)
                nc.vector.tensor_tensor(out=va3, in0=x3[:, :ka, :],
                                        in1=er_b[:, :ka, :], op=OP.mult)
                nc.vector.tensor_tensor(out=va3, in0=va3,
                                        in1=ad_b[:, :ka, :], op=OP.subtract)
                wg = work.tile([128, FREE], BF16, tag="wg", bufs=2)
                for j in range(ka):
                    nc.scalar.activation(out=wg[:, j * DV:(j + 1) * DV],
                                         in_=va[:, j * DV:(j + 1) * DV],
                                         func=AF.Copy,
                                         scale=w_scan[:, t * NJ + j:t * NJ + j + 1])
                nc.vector.tensor_tensor(out=vb3, in0=x3[:, ka:, :],
                                        in1=er_b[:, ka:, :], op=OP.mult)
                nc.vector.tensor_tensor(out=vb3, in0=vb3,
                                        in1=ad_b[:, ka:, :], op=OP.subtract)
                for j in range(ka, NJ):
                    nc.vector.tensor_scalar(
                        out=wg[:, j * DV:(j + 1) * DV],
                        in0=vb[:, (j - ka) * DV:(j - ka + 1) * DV],
                        scalar1=w_scan[:, t * NJ + j:t * NJ + j + 1],
                        scalar2=None, op0=OP.mult)
                nc.vector.tensor_tensor(out=x, in0=x, in1=wg, op=OP.subtract)
                return nxt

            # ---------- schedule ----------
            # upfront: groups for chunk 0+1, loads for chunk 0+1, first wsel.
            for k in range(k_need[1]):
                emit_group(k)
            emit_loads(0)
            emit_loads(1)
            ws_ = emit_wsel(0)
            kdone = k_need[1]
            next_seg = 0
            for ch in range(nchunks):
                # groups for chunk ch+2 (already have <= ch+1)
                gk = list(range(kdone, k_need[ch + 2])) if ch + 2 < nchunks else []
                kdone = k_need[ch + 2] if ch + 2 < nchunks else kdone
                steps = list(range(t_start[ch], t_end[ch]))
                # spread group emission across the chunk's steps
                gpos = {}
                for i, k in enumerate(gk):
                    gpos.setdefault(int((i + 1) * len(steps) / (len(gk) + 1)), []).append(k)
                for si, t in enumerate(steps):
                    nxt = emit_step(t, ws_)
                    if nxt is not None:
                        ws_ = nxt
                    for k in gpos.get(si, []):
                        emit_group(k)
                # loads for chunk ch+2 emitted after its groups
                if ch + 2 < nchunks:
                    emit_loads(ch + 2)
                while next_seg < NSEG and 16 * (next_seg + 1) <= t_end[ch]:
                    emit_seg(next_seg)
                    next_seg += 1
            while next_seg < NSEG:
                emit_seg(next_seg)
                next_seg += 1

            # ---------- final: pred ----------
            pre = pers.tile([128, NCH], F32)
            nc.vector.scalar_tensor_tensor(out=pre, in0=th_sb, scalar=SCALE,
                                           in1=beta_sb, op0=OP.mult,
                                           op1=OP.subtract)
            pred_sb = pers.tile([128, NCH], F32)
            nc.scalar.activation(out=pred_sb, in_=pre, func=AF.Sigmoid)
            # pred_sb[p=32u+i, k] -> pred_d[b=i, t=4k+u]
            nc.sync.dma_start(
                out=pred_d.rearrange("b (k u) -> u b k", u=4),
                in_=pred_sb[:, :])

    return nc


_NC_CACHE = {}


def _get_nc(T=T_FULL):
    if T not in _NC_CACHE:
        n = build_nc(T=T)
        n.compile()
        _NC_CACHE[T] = n
    return _NC_CACHE[T]


def make_inmaps(q_data, qa_data, q_tab, qa_tab, Mk, Mv0, Ws, bs, Wa1, ba1, Wa2,
                ba2, Wd1, bd1, Wd2, bd2, We, be, Wad, bad, T=T_FULL):
    hf = ml_dtypes.bfloat16
    f32 = np.float32
    q_data = np.asarray(q_data)
    qa_data = np.asarray(qa_data)

    mv0p = np.zeros((MP, DV), f32)
    mv0p[:M] = np.asarray(Mv0, f32)
    x0 = mv0p.reshape(4, NJ, DV)[:, None].repeat(BL, 1).reshape(128, FREE).astype(hf)
    isel = np.tile(np.eye(BL, dtype=f32), (4, 1)).astype(hf)  # (128, 32), g-major
    ident = np.eye(128, dtype=f32)

    cat = np.concatenate
    common = {
        "qtab": np.ascontiguousarray(np.asarray(q_tab, f32)),
        "qatab": np.ascontiguousarray(np.asarray(qa_tab, f32)),
        "x0": x0, "isel": isel, "ident": ident,
        "ones": np.ones((1, BL * T), f32).astype(hf),
        "mkt": np.asarray(Mk, f32).T.copy().astype(hf),
        "we1": np.asarray(We, f32)[:128].astype(hf),
        "we2": cat([np.asarray(We, f32)[128:], np.asarray(be, f32)[None, :]], 0).astype(hf),
        "wad1": np.asarray(Wad, f32)[:128].astype(hf),
        "wad2": cat([np.asarray(Wad, f32)[128:], np.asarray(bad, f32)[None, :]], 0).astype(hf),
        "wd1": cat([np.asarray(Wd1, f32), np.asarray(bd1, f32)[None, :]], 0).astype(hf),
        "wd2": cat([np.asarray(Wd2, f32), np.asarray(bd2, f32)[None, :]], 0).astype(hf),
        "ws1": np.asarray(Ws, f32)[:128].astype(hf),
        "ws2": np.asarray(Ws, f32)[128:200].astype(hf),
        "ws3": cat([np.asarray(Ws, f32)[200:], np.asarray(bs, f32)[None, :]], 0).astype(hf),
        "wa1": cat([np.asarray(Wa1, f32), np.asarray(ba1, f32)[None, :]], 0).astype(hf),
        "wa2": cat([np.asarray(Wa2, f32), np.asarray(ba2, f32)[None, :]], 0).astype(hf),
    }
    in_maps = []
    for c in range(NCORES):
        sl = slice(BL * c, BL * (c + 1))
        in_maps.append(dict(
            common,
            qi=np.ascontiguousarray(q_data[sl, :T].T).reshape(-1).astype(np.int32),
            qai=np.ascontiguousarray(qa_data[sl, :T].T).reshape(-1).astype(np.int32)))
    return in_maps


def kernel(**inputs):
    nc = _get_nc(T_FULL)
    in_maps = make_inmaps(**inputs)
    res = run_bass_kernel_spmd(nc, in_maps, core_ids=list(range(NCORES)), trace=False)
    return np.concatenate([res.results[c]["pred"] for c in range(NCORES)], axis=0)
